# revision 21
# baseline (speedup 1.0000x reference)
"""GNO message-passing kernel for Trainium2 (8 NeuronCores, dst-sharded).

Math (matches the reference):
    h  = relu(relu(relu(ea@W1+b1)@W2+b2)@W3+b3)
    w  = (h@W4+b4).reshape(E,16,16)
    msg= einsum('ei,eio->eo', x[src], w)
    agg= segment_mean(msg, dst, N)
    out= x@root + agg + bias

Strategy:
  - Edges are sharded by DESTINATION node range: core k owns nodes
    [k*6250, (k+1)*6250) and all edges pointing into them.  Every node's
    full in-edge set lives on one core, so no cross-core combine exists.
  - Per shard, edges are sorted by dst and padded so no node's run crosses
    a 512-edge tile boundary.  Within a tile every node gets a slot
    (0..<=127, deg-0 nodes included); per-edge slot ids ("rank") ship as a
    bf16 tensor and the segment one-hot matrix is built ON DEVICE with a
    single is_equal tensor_scalar against an iota constant.
  - x[src] is gathered on HOST into tile layout (staged to HBM once), so
    the device does zero per-edge indirect DMA.  MLP layer 1 (8->100) is
    also computed on host (tiny flops, saves a matmul+act per tile).
  - Per 512-edge tile: bf16 MLP layers 2-4 on TensorE (bias-as-extra-
    channel trick for layer 4), broadcast multiply w*x[src] on VectorE,
    then 4 PSUM-accumulated one-hot matmuls aggregate the un-reduced
    [128, (o,i)=256] products per slot; one 256->16 reduce + reciprocal-
    degree multiply (host-precomputed) yields the segment mean, written
    densely into an Internal DRAM accum at flat rows (chunk,slot,tile).
  - Phase 2: for each 128-node chunk, one indirect DMA gathers the chunk's
    agg rows (slot position shipped per node as int32), TensorE adds
    x@root+bias via the ones-row trick, result written to the [6250,16]
    f32 output slice.  Host just concatenates the 8 slices.
"""

import contextlib
import math
import numpy as np
import ml_dtypes

import concourse.bass as bass
import concourse.bacc as bacc
import concourse.mybir as mybir
import concourse.tile as tile
from concourse.bass_utils import run_bass_kernel_spmd

BF16 = ml_dtypes.bfloat16

N_NODES = 50000
N_EDGES = 800000
N_CORES = 8
ETILE = 512
P = 128
NSLICE = N_NODES // N_CORES  # 6250 nodes per core
CH = 8                       # tiles per DMA load chunk
G2 = math.ceil(NSLICE / P)   # phase-2 node chunks (49)


# ----------------------------------------------------------------- host prep

def _pack_shard(counts):
    """Greedy pack of the shard's per-node runs (in node order, deg-0
    included) into 512-edge tiles: a run never crosses a tile boundary and
    each tile holds at most 127 distinct nodes (slot 127 = pad edges).
    Returns (new_start[node], tile_of[node], slot_of[node], n_tiles)."""
    n = len(counts)
    new_start = np.empty(n, np.int64)
    tile_of = np.empty(n, np.int32)
    slot_of = np.empty(n, np.int32)
    nslots = [0]
    pos = 0
    for ln, l in enumerate(counts.tolist()):
        assert l <= ETILE, f"in-degree {l} > {ETILE} unsupported"
        fill = pos % ETILE
        if fill + l > ETILE:
            pos += ETILE - fill
        t = pos // ETILE
        while t >= len(nslots):
            nslots.append(0)
        if nslots[t] >= P - 1:  # tile slot overflow (rare): spill to next
            pos = (t + 1) * ETILE
            t += 1
            nslots.append(0)
        slot_of[ln] = nslots[t]
        nslots[t] += 1
        tile_of[ln] = t
        new_start[ln] = pos
        pos += l
    return new_start, tile_of, slot_of, len(nslots)


def _prep_inputs(x, edge_index, edge_attr, W1, b1, W2, b2, W3, b3, W4, b4,
                 root, bias):
    src_all = np.asarray(edge_index[0], np.int64)
    dst_all = np.asarray(edge_index[1], np.int64)
    attr_all = np.asarray(edge_attr, np.float32)

    order = np.argsort(dst_all, kind="stable")
    src_s = src_all[order]
    attr_s = attr_all[order]
    counts_all = np.bincount(dst_all, minlength=N_NODES)
    run_start_all = np.concatenate([[0], np.cumsum(counts_all)])

    packs = []
    T = 0
    for k in range(N_CORES):
        counts = counts_all[k * NSLICE:(k + 1) * NSLICE]
        new_start, tile_of, slot_of, tk = _pack_shard(counts)
        packs.append((counts, new_start, tile_of, slot_of))
        T = max(T, tk)
    Ep = T * ETILE

    # weights, channel-major layouts (same trick as before: W4 columns in
    # (o,i) order, bias row via a constant-1 extra channel from layer 3)
    W4p = np.asarray(W4, np.float32).reshape(100, 16, 16).transpose(0, 2, 1).reshape(100, 256)
    b4p = np.asarray(b4, np.float32).reshape(16, 16).T.reshape(256)
    W4a = np.concatenate([W4p, b4p[None, :]], axis=0).astype(BF16)  # [101,256]
    roota = np.concatenate([np.asarray(root, np.float32),
                            np.asarray(bias, np.float32)[None, :]], axis=0).astype(BF16)
    W3a = np.concatenate([np.asarray(W3, np.float32),
                          np.zeros((100, 1), np.float32)], axis=1).astype(BF16)
    b3a = np.concatenate([np.asarray(b3, np.float32),
                          np.ones(1, np.float32)]).reshape(101, 1)
    iota = np.ascontiguousarray(
        np.broadcast_to(np.arange(P, dtype=np.float32), (P, P))).astype(BF16)
    xbf = np.asarray(x, np.float32).astype(BF16)
    const = {
        "W2": np.asarray(W2, np.float32).astype(BF16),
        "W3": W3a,
        "W4a": W4a,
        "b2": np.asarray(b2, np.float32).reshape(100, 1),
        "b3": b3a,
        "roota": roota,
        "iota": iota,
    }

    NC = math.ceil(T / CH)
    Tp = NC * CH  # tiles padded to a whole number of load chunks
    in_maps = []
    for k in range(N_CORES):
        counts, new_start, tile_of, slot_of = packs[k]
        lo, hi = run_start_all[k * NSLICE], run_start_all[(k + 1) * NSLICE]
        src_k = src_s[lo:hi]
        attr_k = attr_s[lo:hi]

        nz = counts > 0
        lens = counts[nz]
        tot = int(lens.sum())
        # position of each edge after padding
        within = np.arange(tot) - np.repeat(np.cumsum(lens) - lens, lens)
        new_pos = np.repeat(new_start[nz], lens) + within

        src_p = np.zeros(Tp * ETILE, np.int64)
        attr_p = np.zeros((Tp * ETILE, 8), np.float32)
        rank_p = np.full(Tp * ETILE, P - 1, np.float32)
        src_p[new_pos] = src_k
        attr_p[new_pos] = attr_k
        rank_p[new_pos] = np.repeat(slot_of[nz], lens)

        # layer 1 on host (tiny flops, huge device-instruction savings)
        h1_p = np.maximum(attr_p @ np.asarray(W1, np.float32)
                          + np.asarray(b1, np.float32), 0.0)
        h1T = np.ascontiguousarray(h1_p.T).astype(BF16)  # [100, Tp*512]
        # per-chunk partition-major layouts: [NC, 128, CH, ...]
        rank4 = rank_p.reshape(NC, CH, 4, P).transpose(0, 3, 1, 2)
        # per-(tile,slot) reciprocal in-degree (1.0 on unused slots)
        rec_h = np.ones((Tp, P), np.float32)
        rec_h[tile_of, slot_of] = 1.0 / np.maximum(counts, 1)
        rec4 = rec_h.reshape(NC, CH, 1, P).transpose(0, 3, 1, 2)
        rank4 = np.ascontiguousarray(
            np.concatenate([rank4, rec4], axis=3)).astype(BF16)  # [NC,P,CH,5]
        xg = xbf[src_p]  # [Tp*512, 16]
        xg = np.ascontiguousarray(
            xg.reshape(NC, CH, 4, P, 16).transpose(0, 3, 1, 2, 4))

        # phase-2: accum flat row (chunk, slot, tile-in-chunk) of each node
        accrow = ((tile_of.astype(np.int64) // CH) * (P * CH)
                  + slot_of.astype(np.int64) * CH
                  + tile_of.astype(np.int64) % CH).astype(np.int32)
        nodeidx = np.zeros((G2 * P, 1), np.int32)
        nodeidx[:NSLICE, 0] = accrow
        # static per-chunk read bound used by build_program to overlap
        # phase 2 with phase 1 (must hold on every core)
        for q in range(G2):
            hi_n = min((q + 1) * P, NSLICE)
            lim = _chunk_limit(q, T)
            assert int(accrow[q * P:hi_n].max()) < lim, (q, lim)

        xsl = np.asarray(x[k * NSLICE:(k + 1) * NSLICE], np.float32).T
        xslT = np.ascontiguousarray(
            np.concatenate([xsl, np.ones((1, NSLICE), np.float32)], axis=0)
        ).astype(BF16)  # [17, NSLICE] with ones row for the bias

        in_maps.append(dict(const, h1T=h1T, rank4=rank4, xg=xg,
                            nodeidx=nodeidx, xslT=xslT))
    return in_maps, T


def _chunk_limit(q, T):
    """Static upper bound (exclusive) on accum flat rows read by phase-2
    chunk q: nodes are spread ~uniformly over tiles, margin of 4 tiles."""
    NC = math.ceil(T / CH)
    t_max = min(math.ceil((q + 1) * P / NSLICE * T) + 4, NC * CH)
    return min(math.ceil(t_max / CH), NC) * P * CH


# ------------------------------------------------------------ device program

_PROG_CACHE = {}


def build_program(T, nslice=NSLICE, repeat=1, debug=True):
    key = (T, nslice, repeat, debug)
    if key in _PROG_CACHE:
        return _PROG_CACHE[key]

    f32, bf16, i32 = mybir.dt.float32, mybir.dt.bfloat16, mybir.dt.int32
    NC = math.ceil(T / CH)
    Tp = NC * CH

    nc = bacc.Bacc(None, target_bir_lowering=False, debug=debug)
    h1T = nc.dram_tensor("h1T", [100, Tp * ETILE], bf16, kind="ExternalInput")
    rank4 = nc.dram_tensor("rank4", [NC, P, CH * 5], bf16, kind="ExternalInput")
    xgd = nc.dram_tensor("xg", [NC, P, CH * 4 * 16], bf16, kind="ExternalInput")
    nodeidx = nc.dram_tensor("nodeidx", [G2 * P, 1], i32, kind="ExternalInput")
    xslT = nc.dram_tensor("xslT", [17, nslice], bf16, kind="ExternalInput")
    W2 = nc.dram_tensor("W2", [100, 100], bf16, kind="ExternalInput")
    W3 = nc.dram_tensor("W3", [100, 101], bf16, kind="ExternalInput")
    W4a = nc.dram_tensor("W4a", [101, 256], bf16, kind="ExternalInput")
    b2 = nc.dram_tensor("b2", [100, 1], f32, kind="ExternalInput")
    b3 = nc.dram_tensor("b3", [101, 1], f32, kind="ExternalInput")
    roota = nc.dram_tensor("roota", [17, 16], bf16, kind="ExternalInput")
    iota = nc.dram_tensor("iota", [P, P], bf16, kind="ExternalInput")
    accum = nc.dram_tensor("accum", [Tp * P, 16], f32, kind="Internal")
    out = nc.dram_tensor("out", [nslice, 16], f32, kind="ExternalOutput")

    AT = mybir.ActivationFunctionType
    AX = mybir.AxisListType
    OP = mybir.AluOpType

    with tile.TileContext(nc) as tc, \
         nc.allow_low_precision(reason="bf16 intermediates, fp32 accumulation"):
        with tc.tile_pool(name="consts", bufs=1) as cp, \
             tc.tile_pool(name="loads", bufs=3) as lp, \
             tc.tile_pool(name="work", bufs=3) as wp, \
             tc.tile_pool(name="small", bufs=8) as sp, \
             tc.tile_pool(name="psmlp", bufs=2, space="PSUM") as pm, \
             tc.tile_pool(name="psw", bufs=2, space="PSUM") as pw, \
             tc.tile_pool(name="psagg", bufs=2, space="PSUM") as pa:

            W2sb = cp.tile([100, 100], bf16)
            W3sb = cp.tile([100, 101], bf16)
            W4sb = cp.tile([101, 256], bf16)
            b2sb = cp.tile([100, 1], f32)
            b3sb = cp.tile([101, 1], f32)
            rsb = cp.tile([17, 16], bf16)
            iosb = cp.tile([P, P], bf16)
            xssb = cp.tile([17, nslice], bf16)
            nisb = cp.tile([P, G2], i32)
            for t_sb, t_dr in ((W2sb, W2), (W3sb, W3), (W4sb, W4a),
                               (b2sb, b2), (b3sb, b3), (rsb, roota),
                               (iosb, iota), (xssb, xslT)):
                nc.sync.dma_start(t_sb[:], t_dr[:])
            nc.sync.dma_start(
                nisb[:], nodeidx[:].rearrange("(c p) o -> p (c o)", p=P))

            with (tc.For_i(0, repeat, 1) if repeat > 1
                  else contextlib.nullcontext()):
              for c in range(NC):
                a_sb = lp.tile([100, CH * ETILE], bf16, tag="h1")
                nc.sync.dma_start(a_sb[:], h1T[:, c * CH * ETILE:(c + 1) * CH * ETILE])
                r_sb = lp.tile([P, CH, 5], bf16, tag="rank")
                nc.sync.dma_start(r_sb[:].rearrange("p a b -> p (a b)"), rank4[c])
                x_sb = lp.tile([P, CH, 4, 16], bf16, tag="xg")
                nc.sync.dma_start(x_sb[:].rearrange("p a b d -> p (a b d)"), xgd[c])
                stag = lp.tile([P, CH, 16], f32, tag="stag")

                for j in range(CH):
                    t = c * CH + j
                    ps2 = pm.tile([100, ETILE], f32, tag="mlp")
                    nc.tensor.matmul(ps2[:], lhsT=W2sb[:],
                                     rhs=a_sb[:, j * ETILE:(j + 1) * ETILE],
                                     start=True, stop=True)
                    h2 = wp.tile([100, ETILE], bf16, tag="h2")
                    nc.scalar.activation(h2[:], ps2[:], AT.Relu, bias=b2sb[:, 0:1])
                    ps3 = pm.tile([101, ETILE], f32, tag="mlp")
                    nc.tensor.matmul(ps3[:], lhsT=W3sb[:], rhs=h2[:], start=True, stop=True)
                    h3 = wp.tile([101, ETILE], bf16, tag="h3")
                    nc.scalar.activation(h3[:], ps3[:], AT.Relu, bias=b3sb[:, 0:1])

                    wps4 = pw.tile([P, 4, 256], f32, tag="w")
                    for g in range(4):
                        nc.tensor.matmul(wps4[:, g, :], lhsT=h3[:, g * P:(g + 1) * P],
                                         rhs=W4sb[:], start=True, stop=True)
                    pr = sp.tile([P, 4, 256], bf16, tag="prod")
                    nc.vector.tensor_tensor(
                        out=pr[:].rearrange("p g (o i) -> p g o i", i=16),
                        in0=wps4[:].rearrange("p g (o i) -> p g o i", i=16),
                        in1=x_sb[:, j, :, None, :].to_broadcast([P, 4, 16, 16]),
                        op=OP.mult)
                    oh4 = sp.tile([P, 4, P], bf16, tag="oh")
                    nc.vector.tensor_tensor(
                        out=oh4[:],
                        in0=iosb[:, None, :].to_broadcast([P, 4, P]),
                        in1=r_sb[:, j, 0:4][:, :, None].to_broadcast([P, 4, P]),
                        op=OP.is_equal)
                    ag = pa.tile([P, 256], f32, tag="agg")
                    for g in range(4):
                        nc.tensor.matmul(ag[:], lhsT=oh4[:, g, :], rhs=pr[:, g, :],
                                         start=(g == 0), stop=(g == 3))
                    sums = sp.tile([P, 16], f32, tag="sums")
                    nc.vector.reduce_sum(
                        out=sums[:],
                        in_=ag[:].rearrange("p (o i) -> p o i", i=16),
                        axis=AX.X)
                    nc.vector.tensor_tensor(
                        out=stag[:, j, :], in0=sums[:],
                        in1=r_sb[:, j, 4:5].to_broadcast([P, 16]), op=OP.mult)
                nc.sync.dma_start(
                    accum[c * CH * P:(c + 1) * CH * P, :].rearrange(
                        "(p a) o -> p (a o)", p=P),
                    stag[:].rearrange("p a o -> p (a o)"))

              # phase 2: x@root + bias + agg, in node order
              for q in range(G2):
                n0 = q * P
                w = min(P, nslice - n0)
                agsb = wp.tile([P, 16], f32, tag="agsb")
                nc.gpsimd.indirect_dma_start(
                    out=agsb[:], out_offset=None,
                    in_=accum[0:_chunk_limit(q, T), :],
                    in_offset=bass.IndirectOffsetOnAxis(ap=nisb[:, q:q + 1], axis=0))
                rp = pa.tile([P, 16], f32, tag="agg")
                nc.tensor.matmul(rp[:w, :], lhsT=xssb[:, n0:n0 + w], rhs=rsb[:],
                                 start=True, stop=True)
                ro = wp.tile([P, 16], f32, tag="ro")
                nc.vector.tensor_tensor(out=ro[:w, :], in0=rp[:w, :],
                                        in1=agsb[:w, :], op=OP.add)
                nc.sync.dma_start(out[n0:n0 + w, :], ro[:w, :])

    nc.compile()
    _PROG_CACHE[key] = nc
    return nc


# ------------------------------------------------------------------- driver

def _run(inputs, trace=False):
    in_maps, T = _prep_inputs(**inputs)
    nc = build_program(T)
    res = run_bass_kernel_spmd(nc, in_maps, list(range(N_CORES)), trace=trace)
    out = np.concatenate([r["out"] for r in res.results], axis=0)
    return np.ascontiguousarray(out, dtype=np.float32), res


def kernel(**inputs) -> np.ndarray:
    out, _ = _run(inputs, trace=False)
    return out


# revision 25
# speedup vs baseline: 1.0170x; 1.0170x over previous
"""GNO message-passing kernel for Trainium2 (8 NeuronCores, dst-sharded).

Math (matches the reference):
    h  = relu(relu(relu(ea@W1+b1)@W2+b2)@W3+b3)
    w  = (h@W4+b4).reshape(E,16,16)
    msg= einsum('ei,eio->eo', x[src], w)
    agg= segment_mean(msg, dst, N)
    out= x@root + agg + bias

Strategy:
  - Edges are sharded by DESTINATION node range: core k owns nodes
    [k*6250, (k+1)*6250) and all edges pointing into them.  Every node's
    full in-edge set lives on one core, so no cross-core combine exists.
  - Per shard, edges are sorted by dst and padded so no node's run crosses
    a 512-edge tile boundary.  Within a tile every node gets a slot
    (0..<=127, deg-0 nodes included); per-edge slot ids ("rank") ship as a
    bf16 tensor and the segment one-hot matrix is built ON DEVICE with a
    single is_equal tensor_scalar against an iota constant.
  - x[src] is gathered on HOST into tile layout (staged to HBM once), so
    the device does zero per-edge indirect DMA.  MLP layer 1 (8->100) is
    also computed on host (tiny flops, saves a matmul+act per tile).
  - Per 512-edge tile: bf16 MLP layers 2-4 on TensorE (bias-as-extra-
    channel trick for layer 4), broadcast multiply w*x[src] on VectorE,
    then 4 PSUM-accumulated one-hot matmuls aggregate the un-reduced
    [128, (o,i)=256] products per slot; one 256->16 reduce + reciprocal-
    degree multiply (host-precomputed) yields the segment mean, written
    densely into an Internal DRAM accum at flat rows (chunk,slot,tile).
  - Phase 2: for each 128-node chunk, one indirect DMA gathers the chunk's
    agg rows (slot position shipped per node as int32), TensorE adds
    x@root+bias via the ones-row trick, result written to the [6250,16]
    f32 output slice.  Host just concatenates the 8 slices.
"""

import contextlib
import math
import numpy as np
import ml_dtypes

import concourse.bass as bass
import concourse.bacc as bacc
import concourse.mybir as mybir
import concourse.tile as tile
from concourse.bass_utils import run_bass_kernel_spmd

BF16 = ml_dtypes.bfloat16

N_NODES = 50000
N_EDGES = 800000
N_CORES = 8
ETILE = 512
P = 128
NSLICE = N_NODES // N_CORES  # 6250 nodes per core
CH = 8                       # tiles per DMA load chunk
G2 = math.ceil(NSLICE / P)   # phase-2 node chunks (49)


# ----------------------------------------------------------------- host prep

def _pack_shard(counts):
    """Greedy pack of the shard's per-node runs (in node order, deg-0
    included) into 512-edge tiles: a run never crosses a tile boundary and
    each tile holds at most 127 distinct nodes (slot 127 = pad edges).
    Returns (new_start[node], tile_of[node], slot_of[node], n_tiles)."""
    n = len(counts)
    new_start = np.empty(n, np.int64)
    tile_of = np.empty(n, np.int32)
    slot_of = np.empty(n, np.int32)
    nslots = [0]
    pos = 0
    for ln, l in enumerate(counts.tolist()):
        assert l <= ETILE, f"in-degree {l} > {ETILE} unsupported"
        fill = pos % ETILE
        if fill + l > ETILE:
            pos += ETILE - fill
        t = pos // ETILE
        while t >= len(nslots):
            nslots.append(0)
        if nslots[t] >= P - 1:  # tile slot overflow (rare): spill to next
            pos = (t + 1) * ETILE
            t += 1
            nslots.append(0)
        slot_of[ln] = nslots[t]
        nslots[t] += 1
        tile_of[ln] = t
        new_start[ln] = pos
        pos += l
    return new_start, tile_of, slot_of, len(nslots)


def _prep_inputs(x, edge_index, edge_attr, W1, b1, W2, b2, W3, b3, W4, b4,
                 root, bias):
    src_all = np.asarray(edge_index[0], np.int64)
    dst_all = np.asarray(edge_index[1], np.int64)
    attr_all = np.asarray(edge_attr, np.float32)

    order = np.argsort(dst_all, kind="stable")
    src_s = src_all[order]
    attr_s = attr_all[order]
    counts_all = np.bincount(dst_all, minlength=N_NODES)
    run_start_all = np.concatenate([[0], np.cumsum(counts_all)])

    packs = []
    T = 0
    for k in range(N_CORES):
        counts = counts_all[k * NSLICE:(k + 1) * NSLICE]
        new_start, tile_of, slot_of, tk = _pack_shard(counts)
        packs.append((counts, new_start, tile_of, slot_of))
        T = max(T, tk)
    Ep = T * ETILE

    # weights, channel-major layouts (same trick as before: W4 columns in
    # (o,i) order, bias row via a constant-1 extra channel from layer 3)
    W4p = np.asarray(W4, np.float32).reshape(100, 16, 16).transpose(0, 2, 1).reshape(100, 256)
    b4p = np.asarray(b4, np.float32).reshape(16, 16).T.reshape(256)
    W4a = np.concatenate([W4p, b4p[None, :]], axis=0).astype(BF16)  # [101,256]
    roota = np.concatenate([np.asarray(root, np.float32),
                            np.asarray(bias, np.float32)[None, :]], axis=0).astype(BF16)
    W3a = np.concatenate([np.asarray(W3, np.float32),
                          np.zeros((100, 1), np.float32)], axis=1).astype(BF16)
    b3a = np.concatenate([np.asarray(b3, np.float32),
                          np.ones(1, np.float32)]).reshape(101, 1)
    iota = np.ascontiguousarray(
        np.broadcast_to(np.arange(P, dtype=np.float32), (P, P))).astype(BF16)
    xbf = np.asarray(x, np.float32).astype(BF16)
    const = {
        "W2": np.asarray(W2, np.float32).astype(BF16),
        "W3": W3a,
        "W4a": W4a,
        "b2": np.asarray(b2, np.float32).reshape(100, 1),
        "b3": b3a,
        "roota": roota,
        "iota": iota,
    }

    NC = math.ceil(T / CH)
    Tp = NC * CH  # tiles padded to a whole number of load chunks
    in_maps = []
    for k in range(N_CORES):
        counts, new_start, tile_of, slot_of = packs[k]
        lo, hi = run_start_all[k * NSLICE], run_start_all[(k + 1) * NSLICE]
        src_k = src_s[lo:hi]
        attr_k = attr_s[lo:hi]

        nz = counts > 0
        lens = counts[nz]
        tot = int(lens.sum())
        # position of each edge after padding
        within = np.arange(tot) - np.repeat(np.cumsum(lens) - lens, lens)
        new_pos = np.repeat(new_start[nz], lens) + within

        src_p = np.zeros(Tp * ETILE, np.int64)
        attr_p = np.zeros((Tp * ETILE, 8), np.float32)
        rank_p = np.full(Tp * ETILE, P - 1, np.float32)
        src_p[new_pos] = src_k
        attr_p[new_pos] = attr_k
        rank_p[new_pos] = np.repeat(slot_of[nz], lens)

        # layer 1 on host (tiny flops, huge device-instruction savings)
        h1_p = np.maximum(attr_p @ np.asarray(W1, np.float32)
                          + np.asarray(b1, np.float32), 0.0)
        h1T = np.ascontiguousarray(h1_p.T).astype(BF16)  # [100, Tp*512]
        # per-chunk partition-major layouts: [NC, 128, CH, ...]
        rank4 = rank_p.reshape(NC, CH, 4, P).transpose(0, 3, 1, 2)
        # per-(tile,slot) reciprocal in-degree (1.0 on unused slots)
        rec_h = np.ones((Tp, P), np.float32)
        rec_h[tile_of, slot_of] = 1.0 / np.maximum(counts, 1)
        rec4 = rec_h.reshape(NC, CH, 1, P).transpose(0, 3, 1, 2)
        rank4 = np.ascontiguousarray(
            np.concatenate([rank4, rec4], axis=3)).astype(BF16)  # [NC,P,CH,5]
        xg = xbf[src_p]  # [Tp*512, 16]
        xg = np.ascontiguousarray(
            xg.reshape(NC, CH, 4, P, 16).transpose(0, 3, 1, 2, 4))

        # phase-2: accum flat row (chunk, slot, tile-in-chunk) of each node
        accrow = ((tile_of.astype(np.int64) // CH) * (P * CH)
                  + slot_of.astype(np.int64) * CH
                  + tile_of.astype(np.int64) % CH).astype(np.int32)
        nodeidx = np.zeros((G2 * P, 1), np.int32)
        nodeidx[:NSLICE, 0] = accrow
        # static per-chunk read bound used by build_program to overlap
        # phase 2 with phase 1 (must hold on every core)
        for q in range(G2):
            hi_n = min((q + 1) * P, NSLICE)
            lim = _chunk_limit(q, T)
            assert int(accrow[q * P:hi_n].max()) < lim, (q, lim)

        xsl = np.asarray(x[k * NSLICE:(k + 1) * NSLICE], np.float32).T
        xslT = np.ascontiguousarray(
            np.concatenate([xsl, np.ones((1, NSLICE), np.float32)], axis=0)
        ).astype(BF16)  # [17, NSLICE] with ones row for the bias

        in_maps.append(dict(const, h1T=h1T, rank4=rank4, xg=xg,
                            nodeidx=nodeidx, xslT=xslT))
    return in_maps, T


def _chunk_limit(q, T):
    """Static upper bound (exclusive) on accum flat rows read by phase-2
    chunk q: nodes are spread ~uniformly over tiles, margin of 4 tiles."""
    NC = math.ceil(T / CH)
    t_max = min(math.ceil((q + 1) * P / NSLICE * T) + 4, NC * CH)
    return min(math.ceil(t_max / CH), NC) * P * CH


# ------------------------------------------------------------ device program

_PROG_CACHE = {}


def build_program(T, nslice=NSLICE, repeat=1, debug=True):
    key = (T, nslice, repeat, debug)
    if key in _PROG_CACHE:
        return _PROG_CACHE[key]

    f32, bf16, i32 = mybir.dt.float32, mybir.dt.bfloat16, mybir.dt.int32
    NC = math.ceil(T / CH)
    Tp = NC * CH

    nc = bacc.Bacc(None, target_bir_lowering=False, debug=debug)
    h1T = nc.dram_tensor("h1T", [100, Tp * ETILE], bf16, kind="ExternalInput")
    rank4 = nc.dram_tensor("rank4", [NC, P, CH * 5], bf16, kind="ExternalInput")
    xgd = nc.dram_tensor("xg", [NC, P, CH * 4 * 16], bf16, kind="ExternalInput")
    nodeidx = nc.dram_tensor("nodeidx", [G2 * P, 1], i32, kind="ExternalInput")
    xslT = nc.dram_tensor("xslT", [17, nslice], bf16, kind="ExternalInput")
    W2 = nc.dram_tensor("W2", [100, 100], bf16, kind="ExternalInput")
    W3 = nc.dram_tensor("W3", [100, 101], bf16, kind="ExternalInput")
    W4a = nc.dram_tensor("W4a", [101, 256], bf16, kind="ExternalInput")
    b2 = nc.dram_tensor("b2", [100, 1], f32, kind="ExternalInput")
    b3 = nc.dram_tensor("b3", [101, 1], f32, kind="ExternalInput")
    roota = nc.dram_tensor("roota", [17, 16], bf16, kind="ExternalInput")
    iota = nc.dram_tensor("iota", [P, P], bf16, kind="ExternalInput")
    accum = nc.dram_tensor("accum", [Tp * P, 16], f32, kind="Internal")
    out = nc.dram_tensor("out", [nslice, 16], f32, kind="ExternalOutput")

    AT = mybir.ActivationFunctionType
    AX = mybir.AxisListType
    OP = mybir.AluOpType

    with tile.TileContext(nc) as tc, \
         nc.allow_low_precision(reason="bf16 intermediates, fp32 accumulation"):
        with tc.tile_pool(name="consts", bufs=1) as cp, \
             tc.tile_pool(name="loads", bufs=3) as lp, \
             tc.tile_pool(name="work", bufs=3) as wp, \
             tc.tile_pool(name="small", bufs=8) as sp, \
             tc.tile_pool(name="psmlp", bufs=2, space="PSUM") as pm, \
             tc.tile_pool(name="psw", bufs=2, space="PSUM") as pw, \
             tc.tile_pool(name="psagg", bufs=2, space="PSUM") as pa:

            W2sb = cp.tile([100, 100], bf16)
            W3sb = cp.tile([100, 101], bf16)
            W4sb = cp.tile([101, 256], bf16)
            b2sb = cp.tile([100, 1], f32)
            b3sb = cp.tile([101, 1], f32)
            rsb = cp.tile([17, 16], bf16)
            iosb = cp.tile([P, P], bf16)
            xssb = cp.tile([17, nslice], bf16)
            nisb = cp.tile([P, G2], i32)
            for t_sb, t_dr in ((W2sb, W2), (W3sb, W3), (W4sb, W4a),
                               (b2sb, b2), (b3sb, b3), (rsb, roota),
                               (iosb, iota), (xssb, xslT)):
                nc.sync.dma_start(t_sb[:], t_dr[:])
            nc.sync.dma_start(
                nisb[:], nodeidx[:].rearrange("(c p) o -> p (c o)", p=P))

            with (tc.For_i(0, repeat, 1) if repeat > 1
                  else contextlib.nullcontext()):
              for c in range(NC):
                a_sb = lp.tile([100, CH * ETILE], bf16, tag="h1")
                nc.sync.dma_start(a_sb[:], h1T[:, c * CH * ETILE:(c + 1) * CH * ETILE])
                r_sb = lp.tile([P, CH, 5], bf16, tag="rank")
                nc.sync.dma_start(r_sb[:].rearrange("p a b -> p (a b)"), rank4[c])
                x_sb = lp.tile([P, CH, 4, 16], bf16, tag="xg")
                nc.sync.dma_start(x_sb[:].rearrange("p a b d -> p (a b d)"), xgd[c])
                stag = lp.tile([P, CH, 16], f32, tag="stag")

                for j in range(CH):
                    t = c * CH + j
                    ps2 = pm.tile([100, ETILE], f32, tag="mlp")
                    nc.tensor.matmul(ps2[:], lhsT=W2sb[:],
                                     rhs=a_sb[:, j * ETILE:(j + 1) * ETILE],
                                     start=True, stop=True)
                    h2 = wp.tile([100, ETILE], bf16, tag="h2")
                    nc.scalar.activation(h2[:], ps2[:], AT.Relu, bias=b2sb[:, 0:1])
                    ps3 = pm.tile([101, ETILE], f32, tag="mlp")
                    nc.tensor.matmul(ps3[:], lhsT=W3sb[:], rhs=h2[:], start=True, stop=True)
                    h3 = wp.tile([101, ETILE], bf16, tag="h3")
                    nc.scalar.activation(h3[:], ps3[:], AT.Relu, bias=b3sb[:, 0:1])

                    wps4 = pw.tile([P, 4, 256], f32, tag="w")
                    for g in range(4):
                        nc.tensor.matmul(wps4[:, g, :], lhsT=h3[:, g * P:(g + 1) * P],
                                         rhs=W4sb[:], start=True, stop=True)
                    pr = sp.tile([P, 4, 256], bf16, tag="prod")
                    nc.vector.tensor_tensor(
                        out=pr[:].rearrange("p g (o i) -> p g o i", i=16),
                        in0=wps4[:].rearrange("p g (o i) -> p g o i", i=16),
                        in1=x_sb[:, j, :, None, :].to_broadcast([P, 4, 16, 16]),
                        op=OP.mult)
                    oh4 = sp.tile([P, 4, P], bf16, tag="oh")
                    nc.vector.tensor_tensor(
                        out=oh4[:],
                        in0=iosb[:, None, :].to_broadcast([P, 4, P]),
                        in1=r_sb[:, j, 0:4][:, :, None].to_broadcast([P, 4, P]),
                        op=OP.is_equal)
                    ag = pa.tile([P, 256], f32, tag="agg")
                    for g in range(4):
                        nc.tensor.matmul(ag[:], lhsT=oh4[:, g, :], rhs=pr[:, g, :],
                                         start=(g == 0), stop=(g == 3))
                    sums = sp.tile([P, 16], f32, tag="sums")
                    nc.vector.reduce_sum(
                        out=sums[:],
                        in_=ag[:].rearrange("p (o i) -> p o i", i=16),
                        axis=AX.X)
                    nc.vector.tensor_tensor(
                        out=stag[:, j, :], in0=sums[:],
                        in1=r_sb[:, j, 4:5].to_broadcast([P, 16]), op=OP.mult)
                nc.sync.dma_start(
                    accum[c * CH * P:(c + 1) * CH * P, :].rearrange(
                        "(p a) o -> p (a o)", p=P),
                    stag[:].rearrange("p a o -> p (a o)"))

              # phase 2: x@root + bias + agg, in node order
              for q in range(G2):
                n0 = q * P
                w = min(P, nslice - n0)
                agsb = wp.tile([P, 16], f32, tag="agsb")
                nc.gpsimd.indirect_dma_start(
                    out=agsb[:], out_offset=None,
                    in_=accum[0:_chunk_limit(q, T), :],
                    in_offset=bass.IndirectOffsetOnAxis(ap=nisb[:, q:q + 1], axis=0))
                rp = pa.tile([P, 16], f32, tag="agg")
                nc.tensor.matmul(rp[:w, :], lhsT=xssb[:, n0:n0 + w], rhs=rsb[:],
                                 start=True, stop=True)
                ro = wp.tile([P, 16], f32, tag="ro")
                nc.vector.tensor_tensor(out=ro[:w, :], in0=rp[:w, :],
                                        in1=agsb[:w, :], op=OP.add)
                nc.sync.dma_start(out[n0:n0 + w, :], ro[:w, :])

    nc.compile()
    _PROG_CACHE[key] = nc
    return nc


# ------------------------------------------------------------------- driver

def _run(inputs, trace=False):
    in_maps, T = _prep_inputs(**inputs)
    nc = build_program(T)
    res = run_bass_kernel_spmd(nc, in_maps, list(range(N_CORES)), trace=trace)
    out = np.concatenate([r["out"] for r in res.results], axis=0)
    return np.ascontiguousarray(out, dtype=np.float32), res


def kernel(**inputs) -> np.ndarray:
    out, _ = _run(inputs, trace=False)
    return out


# revision 26
# speedup vs baseline: 1.1746x; 1.1549x over previous
"""GNO message-passing kernel for Trainium2 (8 NeuronCores, dst-sharded).

Math (matches the reference):
    h  = relu(relu(relu(ea@W1+b1)@W2+b2)@W3+b3)
    w  = (h@W4+b4).reshape(E,16,16)
    msg= einsum('ei,eio->eo', x[src], w)
    agg= segment_mean(msg, dst, N)
    out= x@root + agg + bias

Strategy:
  - Edges are sharded by DESTINATION node range: core k owns nodes
    [k*6250, (k+1)*6250) and all edges pointing into them.  Every node's
    full in-edge set lives on one core, so no cross-core combine exists.
  - Per shard, edges are sorted by dst and padded so no node's run crosses
    a 512-edge tile boundary.  Within a tile every node gets a slot
    (0..<=127, deg-0 nodes included); per-edge slot ids ("rank") ship as a
    bf16 tensor and the segment one-hot matrix is built ON DEVICE with a
    single is_equal tensor_scalar against an iota constant.
  - x[src] is gathered on HOST into tile layout (staged to HBM once), so
    the device does zero per-edge indirect DMA.  MLP layer 1 (8->100) is
    also computed on host (tiny flops, saves a matmul+act per tile).
  - Per 512-edge tile: bf16 MLP layers 2-4 on TensorE (bias-as-extra-
    channel trick for layer 4), broadcast multiply w*x[src] on VectorE,
    then 4 PSUM-accumulated one-hot matmuls aggregate the un-reduced
    [128, (o,i)=256] products per slot; one 256->16 reduce + reciprocal-
    degree multiply (host-precomputed) yields the segment mean, written
    densely into an Internal DRAM accum at flat rows (chunk,slot,tile).
  - Phase 2: for each 128-node chunk, one indirect DMA gathers the chunk's
    agg rows (slot position shipped per node as int32), TensorE adds
    x@root+bias via the ones-row trick, result written to the [6250,16]
    f32 output slice.  Host just concatenates the 8 slices.
"""

import contextlib
import math
import numpy as np
import ml_dtypes

import concourse.bass as bass
import concourse.bacc as bacc
import concourse.mybir as mybir
import concourse.tile as tile
from concourse.bass_utils import run_bass_kernel_spmd

BF16 = ml_dtypes.bfloat16

N_NODES = 50000
N_EDGES = 800000
N_CORES = 8
ETILE = 512
P = 128
NSLICE = N_NODES // N_CORES  # 6250 nodes per core
CH = 8                       # tiles per DMA load chunk
G2 = math.ceil(NSLICE / P)   # phase-2 node chunks (49)


# ----------------------------------------------------------------- host prep

def _pack_shard(counts):
    """Greedy pack of the shard's per-node runs (in node order, deg-0
    included) into 512-edge tiles: a run never crosses a tile boundary and
    each tile holds at most 127 distinct nodes (slot 127 = pad edges).
    Returns (new_start[node], tile_of[node], slot_of[node], n_tiles)."""
    n = len(counts)
    new_start = np.empty(n, np.int64)
    tile_of = np.empty(n, np.int32)
    slot_of = np.empty(n, np.int32)
    nslots = [0]
    pos = 0
    for ln, l in enumerate(counts.tolist()):
        assert l <= ETILE, f"in-degree {l} > {ETILE} unsupported"
        fill = pos % ETILE
        if fill + l > ETILE:
            pos += ETILE - fill
        t = pos // ETILE
        while t >= len(nslots):
            nslots.append(0)
        if nslots[t] >= P - 1:  # tile slot overflow (rare): spill to next
            pos = (t + 1) * ETILE
            t += 1
            nslots.append(0)
        slot_of[ln] = nslots[t]
        nslots[t] += 1
        tile_of[ln] = t
        new_start[ln] = pos
        pos += l
    return new_start, tile_of, slot_of, len(nslots)


def _prep_inputs(x, edge_index, edge_attr, W1, b1, W2, b2, W3, b3, W4, b4,
                 root, bias):
    src_all = np.asarray(edge_index[0], np.int64)
    dst_all = np.asarray(edge_index[1], np.int64)
    attr_all = np.asarray(edge_attr, np.float32)

    order = np.argsort(dst_all, kind="stable")
    src_s = src_all[order]
    attr_s = attr_all[order]
    counts_all = np.bincount(dst_all, minlength=N_NODES)
    run_start_all = np.concatenate([[0], np.cumsum(counts_all)])

    packs = []
    T = 0
    for k in range(N_CORES):
        counts = counts_all[k * NSLICE:(k + 1) * NSLICE]
        new_start, tile_of, slot_of, tk = _pack_shard(counts)
        packs.append((counts, new_start, tile_of, slot_of))
        T = max(T, tk)
    Ep = T * ETILE

    # weights, channel-major layouts (same trick as before: W4 columns in
    # (o,i) order, bias row via a constant-1 extra channel from layer 3)
    W4p = np.asarray(W4, np.float32).reshape(100, 16, 16).transpose(0, 2, 1).reshape(100, 256)
    b4p = np.asarray(b4, np.float32).reshape(16, 16).T.reshape(256)
    W4a = np.concatenate([W4p, b4p[None, :]], axis=0).astype(BF16)  # [101,256]
    roota = np.concatenate([np.asarray(root, np.float32),
                            np.asarray(bias, np.float32)[None, :]], axis=0).astype(BF16)
    W3a = np.concatenate([np.asarray(W3, np.float32),
                          np.zeros((100, 1), np.float32)], axis=1).astype(BF16)
    b3a = np.concatenate([np.asarray(b3, np.float32),
                          np.ones(1, np.float32)]).reshape(101, 1)
    iota = np.ascontiguousarray(
        np.broadcast_to(np.arange(P, dtype=np.float32), (P, P))).astype(BF16)
    xbf = np.asarray(x, np.float32).astype(BF16)
    const = {
        "W2": np.asarray(W2, np.float32).astype(BF16),
        "W3": W3a,
        "W4a": W4a,
        "b2": np.asarray(b2, np.float32).reshape(100, 1),
        "b3": b3a,
        "roota": roota,
        "iota": iota,
    }

    NC = math.ceil(T / CH)
    Tp = NC * CH  # tiles padded to a whole number of load chunks
    in_maps = []
    for k in range(N_CORES):
        counts, new_start, tile_of, slot_of = packs[k]
        lo, hi = run_start_all[k * NSLICE], run_start_all[(k + 1) * NSLICE]
        src_k = src_s[lo:hi]
        attr_k = attr_s[lo:hi]

        nz = counts > 0
        lens = counts[nz]
        tot = int(lens.sum())
        # position of each edge after padding
        within = np.arange(tot) - np.repeat(np.cumsum(lens) - lens, lens)
        new_pos = np.repeat(new_start[nz], lens) + within

        src_p = np.zeros(Tp * ETILE, np.int64)
        attr_p = np.zeros((Tp * ETILE, 8), np.float32)
        rank_p = np.full(Tp * ETILE, P - 1, np.float32)
        src_p[new_pos] = src_k
        attr_p[new_pos] = attr_k
        rank_p[new_pos] = np.repeat(slot_of[nz], lens)

        # layer 1 on host (tiny flops, huge device-instruction savings)
        h1_p = np.maximum(attr_p @ np.asarray(W1, np.float32)
                          + np.asarray(b1, np.float32), 0.0)
        h1T = np.ascontiguousarray(h1_p.T).astype(BF16)  # [100, Tp*512]
        # per-chunk partition-major layouts: [NC, 128, CH, ...]
        rank4 = np.ascontiguousarray(
            rank_p.reshape(NC, CH, 4, P).transpose(0, 3, 1, 2)).astype(BF16)
        # fold the segment-mean division into the gathered x: every edge's
        # message is scaled by 1/deg(dst) so the aggregation sum IS the
        # mean.  Pad edges get scale 0.
        rec_e = np.zeros(Tp * ETILE, np.float32)
        rec_e[new_pos] = np.repeat(1.0 / lens, lens)
        xg = xbf[src_p].astype(np.float32) * rec_e[:, None]  # [Tp*512, 16]
        xg = np.ascontiguousarray(
            xg.astype(BF16).reshape(NC, CH, 4, P, 16).transpose(0, 3, 1, 2, 4))

        # phase-2: accum flat row (chunk, slot, tile-in-chunk) of each node
        accrow = ((tile_of.astype(np.int64) // CH) * (P * CH)
                  + slot_of.astype(np.int64) * CH
                  + tile_of.astype(np.int64) % CH).astype(np.int32)
        nodeidx = np.zeros((G2 * P, 1), np.int32)
        nodeidx[:NSLICE, 0] = accrow
        # static per-chunk read bound used by build_program to overlap
        # phase 2 with phase 1 (must hold on every core)
        for q in range(G2):
            hi_n = min((q + 1) * P, NSLICE)
            lim = _chunk_limit(q, T)
            assert int(accrow[q * P:hi_n].max()) < lim, (q, lim)

        xsl = np.asarray(x[k * NSLICE:(k + 1) * NSLICE], np.float32).T
        xslT = np.ascontiguousarray(
            np.concatenate([xsl, np.ones((1, NSLICE), np.float32)], axis=0)
        ).astype(BF16)  # [17, NSLICE] with ones row for the bias

        in_maps.append(dict(const, h1T=h1T, rank4=rank4, xg=xg,
                            nodeidx=nodeidx, xslT=xslT))
    return in_maps, T


def _chunk_limit(q, T):
    """Static upper bound (exclusive) on accum flat rows read by phase-2
    chunk q: nodes are spread ~uniformly over tiles, margin of 4 tiles."""
    NC = math.ceil(T / CH)
    t_max = min(math.ceil((q + 1) * P / NSLICE * T) + 4, NC * CH)
    return min(math.ceil(t_max / CH), NC) * P * CH


# ------------------------------------------------------------ device program

_PROG_CACHE = {}


def build_program(T, nslice=NSLICE, repeat=1, debug=True):
    key = (T, nslice, repeat, debug)
    if key in _PROG_CACHE:
        return _PROG_CACHE[key]

    f32, bf16, i32 = mybir.dt.float32, mybir.dt.bfloat16, mybir.dt.int32
    NC = math.ceil(T / CH)
    Tp = NC * CH

    nc = bacc.Bacc(None, target_bir_lowering=False, debug=debug)
    h1T = nc.dram_tensor("h1T", [100, Tp * ETILE], bf16, kind="ExternalInput")
    rank4 = nc.dram_tensor("rank4", [NC, P, CH * 4], bf16, kind="ExternalInput")
    xgd = nc.dram_tensor("xg", [NC, P, CH * 4 * 16], bf16, kind="ExternalInput")
    nodeidx = nc.dram_tensor("nodeidx", [G2 * P, 1], i32, kind="ExternalInput")
    xslT = nc.dram_tensor("xslT", [17, nslice], bf16, kind="ExternalInput")
    W2 = nc.dram_tensor("W2", [100, 100], bf16, kind="ExternalInput")
    W3 = nc.dram_tensor("W3", [100, 101], bf16, kind="ExternalInput")
    W4a = nc.dram_tensor("W4a", [101, 256], bf16, kind="ExternalInput")
    b2 = nc.dram_tensor("b2", [100, 1], f32, kind="ExternalInput")
    b3 = nc.dram_tensor("b3", [101, 1], f32, kind="ExternalInput")
    roota = nc.dram_tensor("roota", [17, 16], bf16, kind="ExternalInput")
    iota = nc.dram_tensor("iota", [P, P], bf16, kind="ExternalInput")
    accum = nc.dram_tensor("accum", [Tp * P, 16], f32, kind="Internal")
    out = nc.dram_tensor("out", [nslice, 16], f32, kind="ExternalOutput")

    AT = mybir.ActivationFunctionType
    AX = mybir.AxisListType
    OP = mybir.AluOpType

    with tile.TileContext(nc) as tc, \
         nc.allow_low_precision(reason="bf16 intermediates, fp32 accumulation"):
        with tc.tile_pool(name="consts", bufs=1) as cp, \
             tc.tile_pool(name="loads", bufs=3) as lp, \
             tc.tile_pool(name="work", bufs=3) as wp, \
             tc.tile_pool(name="small", bufs=8) as sp, \
             tc.tile_pool(name="psmlp", bufs=2, space="PSUM") as pm, \
             tc.tile_pool(name="psw", bufs=2, space="PSUM") as pw, \
             tc.tile_pool(name="psagg", bufs=2, space="PSUM") as pa:

            W2sb = cp.tile([100, 100], bf16)
            W3sb = cp.tile([100, 101], bf16)
            W4sb = cp.tile([101, 256], bf16)
            b2sb = cp.tile([100, 1], f32)
            b3sb = cp.tile([101, 1], f32)
            rsb = cp.tile([17, 16], bf16)
            iosb = cp.tile([P, P], bf16)
            xssb = cp.tile([17, nslice], bf16)
            nisb = cp.tile([P, G2], i32)
            for t_sb, t_dr in ((W2sb, W2), (W3sb, W3), (W4sb, W4a),
                               (b2sb, b2), (b3sb, b3), (rsb, roota),
                               (iosb, iota), (xssb, xslT)):
                nc.sync.dma_start(t_sb[:], t_dr[:])
            nc.sync.dma_start(
                nisb[:], nodeidx[:].rearrange("(c p) o -> p (c o)", p=P))

            with (tc.For_i(0, repeat, 1) if repeat > 1
                  else contextlib.nullcontext()):
              for c in range(NC):
                a_sb = lp.tile([100, CH * ETILE], bf16, tag="h1")
                nc.sync.dma_start(a_sb[:], h1T[:, c * CH * ETILE:(c + 1) * CH * ETILE])
                r_sb = lp.tile([P, CH, 4], bf16, tag="rank")
                nc.sync.dma_start(r_sb[:].rearrange("p a b -> p (a b)"), rank4[c])
                x_sb = lp.tile([P, CH, 4, 16], bf16, tag="xg")
                nc.sync.dma_start(x_sb[:].rearrange("p a b d -> p (a b d)"), xgd[c])
                stag = lp.tile([P, CH, 16], f32, tag="stag")

                for j in range(CH):
                    t = c * CH + j
                    ps2 = pm.tile([100, ETILE], f32, tag="mlp")
                    nc.tensor.matmul(ps2[:], lhsT=W2sb[:],
                                     rhs=a_sb[:, j * ETILE:(j + 1) * ETILE],
                                     start=True, stop=True)
                    h2 = wp.tile([100, ETILE], bf16, tag="h2")
                    nc.scalar.activation(h2[:], ps2[:], AT.Relu, bias=b2sb[:, 0:1])
                    ps3 = pm.tile([101, ETILE], f32, tag="mlp")
                    nc.tensor.matmul(ps3[:], lhsT=W3sb[:], rhs=h2[:], start=True, stop=True)
                    h3 = wp.tile([101, ETILE], bf16, tag="h3")
                    nc.scalar.activation(h3[:], ps3[:], AT.Relu, bias=b3sb[:, 0:1])

                    wps4 = pw.tile([P, 4, 256], f32, tag="w")
                    for g in range(4):
                        nc.tensor.matmul(wps4[:, g, :], lhsT=h3[:, g * P:(g + 1) * P],
                                         rhs=W4sb[:], start=True, stop=True)
                    pr = sp.tile([P, 4, 256], bf16, tag="prod")
                    nc.vector.tensor_tensor(
                        out=pr[:].rearrange("p g (o i) -> p g o i", i=16),
                        in0=wps4[:].rearrange("p g (o i) -> p g o i", i=16),
                        in1=x_sb[:, j, :, None, :].to_broadcast([P, 4, 16, 16]),
                        op=OP.mult)
                    oh4 = sp.tile([P, 4, P], bf16, tag="oh")
                    nc.vector.tensor_tensor(
                        out=oh4[:],
                        in0=iosb[:, None, :].to_broadcast([P, 4, P]),
                        in1=r_sb[:, j, :][:, :, None].to_broadcast([P, 4, P]),
                        op=OP.is_equal)
                    ag = pa.tile([P, 256], f32, tag="agg")
                    for g in range(4):
                        nc.tensor.matmul(ag[:], lhsT=oh4[:, g, :], rhs=pr[:, g, :],
                                         start=(g == 0), stop=(g == 3))
                    nc.vector.reduce_sum(
                        out=stag[:, j, :],
                        in_=ag[:].rearrange("p (o i) -> p o i", i=16),
                        axis=AX.X)
                nc.sync.dma_start(
                    accum[c * CH * P:(c + 1) * CH * P, :].rearrange(
                        "(p a) o -> p (a o)", p=P),
                    stag[:].rearrange("p a o -> p (a o)"))

              # phase 2: x@root + bias + agg, in node order
              for q in range(G2):
                n0 = q * P
                w = min(P, nslice - n0)
                agsb = wp.tile([P, 16], f32, tag="agsb")
                nc.gpsimd.indirect_dma_start(
                    out=agsb[:], out_offset=None,
                    in_=accum[0:_chunk_limit(q, T), :],
                    in_offset=bass.IndirectOffsetOnAxis(ap=nisb[:, q:q + 1], axis=0))
                rp = pa.tile([P, 16], f32, tag="agg")
                nc.tensor.matmul(rp[:w, :], lhsT=xssb[:, n0:n0 + w], rhs=rsb[:],
                                 start=True, stop=True)
                ro = wp.tile([P, 16], f32, tag="ro")
                nc.vector.tensor_tensor(out=ro[:w, :], in0=rp[:w, :],
                                        in1=agsb[:w, :], op=OP.add)
                nc.sync.dma_start(out[n0:n0 + w, :], ro[:w, :])

    nc.compile()
    _PROG_CACHE[key] = nc
    return nc


# ------------------------------------------------------------------- driver

def _run(inputs, trace=False):
    in_maps, T = _prep_inputs(**inputs)
    nc = build_program(T)
    res = run_bass_kernel_spmd(nc, in_maps, list(range(N_CORES)), trace=trace)
    out = np.concatenate([r["out"] for r in res.results], axis=0)
    return np.ascontiguousarray(out, dtype=np.float32), res


def kernel(**inputs) -> np.ndarray:
    out, _ = _run(inputs, trace=False)
    return out
